# revision 8
# baseline (speedup 1.0000x reference)
"""Adaptive-softmax loss (nn_AdaptiveLoss) on 8 trn2 NeuronCores.

Strategy: tensor-parallel over the vocab dimension, 8-way. Each core owns
1/8 of the shortlist head columns and 1/8 of each tail cluster's output
rows. Per core:

  - computes cluster hidden states h_g = x @ proj_g.T (replicated, small)
    with fp8 DoubleRow matmuls; h0/h1 re-quantized to fp8, h2/h3 to bf16,
  - computes its slice of every group's logits (fp8 DoubleRow where the
    contraction depth allows pairing, bf16 for the K<=128 clusters),
  - exp()s the logits on ACT with per-group scale folding the fp8
    scale factors; softmax partials Z_g[b] ride the ACT accumulator
    (clusters) and DVE tensor_scalar accumulators (head/c0),
  - gathers exp(logit) at this core's share of the targets straight out
    of SBUF (gpsimd indirect_copy), takes ln in-loop (the exp+ln combined
    activation table is preloaded so no table thrash),
  - reduces per-row partial loss statistics (numerator, weight sums, Z),
  - one 45KB AllReduce combines the statistics, then every core finishes
    the (cheap) log/normalize arithmetic identically and writes the scalar.

The full [B, VOCAB] log-prob matrix is never materialized anywhere.
"""

import sys

sys.path.insert(0, "/opt/trn_rl_repo")

from contextlib import ExitStack

import ml_dtypes
import numpy as np

import concourse.bass as bass  # noqa: F401  (engine types via nc.*)
import concourse.mybir as mybir
import concourse.tile as tile
from concourse import bacc
from concourse.bass_utils import run_bass_kernel_spmd

BF16 = ml_dtypes.bfloat16
F8 = ml_dtypes.float8_e4m3
F32 = mybir.dt.float32
BF16_DT = mybir.dt.bfloat16
F8_DT = mybir.dt.float8e4
U16 = mybir.dt.uint16

NCORES = 8
B, T, D = 1024, 128, 1024
VOCAB, SHORT = 100000, 10000
CL_SIZES = [10000, 20000, 40000, 20000]
CL_D = [512, 256, 128, 64]
SH_SHARD = SHORT // NCORES                      # 1250
CL_SHARD = [s // NCORES for s in CL_SIZES]      # 1250 2500 5000 2500
GRP_BOUNDS = [0, 10000, 20000, 40000, 80000, 100000]
GRP_SHARD = [SH_SHARD] + CL_SHARD

# per-core concatenated logits layout: [head | links(4) | c0 | c1 | c2 | c3]
OFF_HEAD = 0
OFF_LINK = SH_SHARD                              # 1250
OFF_CL = [1254, 2504, 5004, 10004]
GRP_OFF = [OFF_HEAD] + OFF_CL                    # per-group concat offset
CONCAT = OFF_CL[-1] + CL_SHARD[-1]               # 12504
CONCAT_PAD = 12544
# pad slots gather column 0 (always computed, finite); their tgid==0 makes
# the ownership mask zero the contribution.
PADIDX = 0
RT = 8                                           # row tiles of 128

# fp8 scale factors (folded back out via the exp activation scale)
S_WHEAD = 64.0                                   # head weight prescale
S_PROJ = 32.0                                    # proj prescale -> h scale
S_WOUT = 64.0                                    # c0/c1 out-proj prescale
EXP_TABLE_ID = 6                                 # natural_log_exp_and_others
import os as _os
FP8_ON = not _os.environ.get("ADAK_BF16")


# ----------------------------------------------------------------------------
# device kernel builder
# ----------------------------------------------------------------------------

_CACHE: dict[int, object] = {}


def _build(S: int):
    """Build + compile the SPMD kernel for padded slot count S (multiple of 16)."""
    if S in _CACHE:
        return _CACHE[S]
    SW = S // 16

    nc = bacc.Bacc("TRN2", target_bir_lowering=False, debug=False,
                   num_devices=NCORES)

    MMDT = F8_DT if FP8_ON else BF16_DT
    xt_d = nc.dram_tensor("xt", [D, B], MMDT, kind="ExternalInput")
    projt_d = nc.dram_tensor("projt", [D, sum(CL_D)], MMDT, kind="ExternalInput")
    whead_d = nc.dram_tensor("wheadt", [D, 1254], MMDT, kind="ExternalInput")
    wout0_d = nc.dram_tensor("wout0t", [CL_D[0], CL_SHARD[0]], MMDT,
                             kind="ExternalInput")
    wout1_d = nc.dram_tensor("wout1t", [CL_D[1], CL_SHARD[1]], MMDT,
                             kind="ExternalInput")
    wout2_d = nc.dram_tensor("wout2t", [CL_D[2], CL_SHARD[2]], BF16_DT,
                             kind="ExternalInput")
    wout3_d = nc.dram_tensor("wout3t", [CL_D[3], CL_SHARD[3]], BF16_DT,
                             kind="ExternalInput")
    tix_d = nc.dram_tensor("tgtidx", [128, RT * SW], U16, kind="ExternalInput")
    tgid_d = nc.dram_tensor("tgid", [128, RT, S], BF16_DT, kind="ExternalInput")
    wm_d = nc.dram_tensor("wm", [128, RT, S], BF16_DT, kind="ExternalInput")
    out_d = nc.dram_tensor("out", [1, 1], F32, kind="ExternalOutput")

    EXP = mybir.ActivationFunctionType.Exp
    LN = mybir.ActivationFunctionType.Ln
    ADD = mybir.AluOpType.add
    SUB = mybir.AluOpType.subtract
    MULT = mybir.AluOpType.mult
    ISEQ = mybir.AluOpType.is_equal
    AXX = mybir.AxisListType.X
    DR = mybir.MatmulPerfMode.DoubleRow

    with tile.TileContext(nc) as tc, ExitStack() as ctx:
        sb = ctx.enter_context(tc.tile_pool(name="sb", bufs=1))
        big = ctx.enter_context(tc.tile_pool(name="big", bufs=3))
        ps = ctx.enter_context(tc.tile_pool(name="ps", bufs=2, space="PSUM"))
        dr = ctx.enter_context(tc.tile_pool(name="dr", bufs=1, space="DRAM"))

        # combined exp+ln activation table so EXP and LN interleave with a
        # single table load for the whole kernel
        import os
        if not os.environ.get("ADAK_NO_TABLE_PRELOAD"):
            nc.scalar.add_instruction(mybir.InstLoadActFuncSet(
                name=nc.get_next_instruction_name(),
                act_func_set_id=EXP_TABLE_ID, ins=[], outs=[]))

        # ---- persistent SBUF tensors ----
        xt_sb = sb.tile([128, 8, B], MMDT)             # x.T  [d, b] k-tiled
        whead_sb = sb.tile([128, 8, 1254], MMDT)
        wout0_sb = sb.tile([128, 4, CL_SHARD[0]], MMDT)
        wout1_sb = sb.tile([128, 2, CL_SHARD[1]], MMDT)
        wout2_sb = sb.tile([128, CL_SHARD[2]], BF16_DT)
        wout3_sb = sb.tile([64, CL_SHARD[3]], BF16_DT)
        h0_sb = sb.tile([128, 4, B], MMDT)             # h.T (x S_PROJ)
        h1_sb = sb.tile([128, 2, B], MMDT)
        h2_sb = sb.tile([128, B], BF16_DT)
        h3_sb = sb.tile([64, B], BF16_DT)
        tmpS = sb.tile([128, S], BF16_DT)
        tix_sb = sb.tile([128, RT * SW], U16)
        tgid_sb = sb.tile([128, RT, S], BF16_DT)
        vg3 = sb.tile([128, RT, S], BF16_DT)           # gathered exp(logit)
        wm_sb = sb.tile([128, RT, S], BF16_DT)         # (1-dp)*ownership
        logv3 = sb.tile([128, RT, S], BF16_DT)
        llinkraw = sb.tile([128, RT, 4], F32)          # raw link logits
        zscr = sb.tile([128, 1280], BF16_DT)
        zs = sb.tile([128, RT, 8], F32)   # Z partials per piece, c1/c2/c3
        # AR payload, 11 stats x 8 row-tiles:
        # q: 0 Zh, 1..4 Zc_g, 5 den, 6..9 Wg, 10 numraw
        pay = sb.tile([128, 88], F32)
        rsb = sb.tile([128, 88], F32)
        ones_sb = sb.tile([128, 1], F32)
        out_sb = sb.tile([1, 1], F32)

        pview = pay[:, :].rearrange("p (q r) -> p q r", q=11)
        rview = rsb[:, :].rearrange("p (q r) -> p q r", q=11)

        # ---- input DMAs (order matters: compute-critical tensors first;
        # xt/projt interleaved per k-tile so the h matmuls start early) ----
        pj = sb.tile([128, 8, sum(CL_D)], MMDT)
        xt_r = xt_d.ap().rearrange("(k p) b -> p k b", p=128)
        pj_r = projt_d.ap().rearrange("(k p) c -> p k c", p=128)
        for k in range(8):
            nc.sync.dma_start(out=xt_sb[:, k, :], in_=xt_r[:, k, :])
            nc.sync.dma_start(out=pj[:, k, :], in_=pj_r[:, k, :])
        nc.sync.dma_start(out=whead_sb,
                          in_=whead_d.ap().rearrange("(k p) c -> p k c", p=128))
        nc.sync.dma_start(out=wout0_sb,
                          in_=wout0_d.ap().rearrange("(k p) c -> p k c", p=128))
        nc.sync.dma_start(out=wout1_sb,
                          in_=wout1_d.ap().rearrange("(k p) c -> p k c", p=128))
        nc.sync.dma_start(out=wout2_sb, in_=wout2_d[:])
        nc.sync.dma_start(out=wout3_sb, in_=wout3_d[:])
        nc.sync.dma_start(out=tix_sb, in_=tix_d[:])
        nc.sync.dma_start(out=tgid_sb, in_=tgid_d[:])
        nc.sync.dma_start(out=wm_sb, in_=wm_d[:])

        nc.vector.memset(ones_sb[:, :], 1.0)

        # Prewarm the collectives path: a tiny dummy AllReduce early in the
        # run (nothing in the loop depends on it) absorbs the ~11.5us
        # first-collective ncfw setup and part of the cross-core launch skew,
        # so the real AllReduce at the tail starts almost immediately.
        warm_src = sb.tile([1, 16], F32)
        warm_sb = sb.tile([1, 16], F32)
        warm_in = dr.tile([1, 16], F32)
        warm_out = dr.tile([1, 16], F32, addr_space="Shared")
        nc.vector.memset(warm_src[:, :], 1.0)
        nc.sync.dma_start(out=warm_in, in_=warm_src[:, :])
        nc.gpsimd.collective_compute(
            "AllReduce", ADD, replica_groups=[list(range(NCORES))],
            ins=[warm_in.opt()], outs=[warm_out.opt()])
        nc.sync.dma_start(out=warm_sb, in_=warm_out)
        nc.vector.tensor_copy(zscr[0:1, 0:16], warm_sb[:, :])

        # ---- cluster hidden states h.T (all batch rows, computed locally —
        # a sharded h + AllGather is infeasible: the collectives entry
        # barrier isn't done until ~60-150us into the NEFF) ----
        HT_OFF = [0, 128, 256, 384, 512, 640, 768, 896]
        HT_M = [128, 128, 128, 128, 128, 128, 128, 64]
        for bc in range(2):
            for htile in range(2):
                pst = ps.tile([128, 2048], F32, tag="ps", name=f"hps_{bc}_{htile}")
                for hl in range(4):
                    ht = htile * 4 + hl
                    M = HT_M[ht]
                    if FP8_ON:
                        for kp in range(4):
                            nc.tensor.matmul(
                                pst[0:M, hl * 512:(hl + 1) * 512],
                                pj[:, 2 * kp:2 * kp + 2, HT_OFF[ht]:HT_OFF[ht] + M],
                                xt_sb[:, 2 * kp:2 * kp + 2, bc * 512:(bc + 1) * 512],
                                start=(kp == 0), stop=(kp == 3), perf_mode=DR)
                    else:
                        for k in range(8):
                            nc.tensor.matmul(
                                pst[0:M, hl * 512:(hl + 1) * 512],
                                pj[:, k, HT_OFF[ht]:HT_OFF[ht] + M],
                                xt_sb[:, k, bc * 512:(bc + 1) * 512],
                                start=(k == 0), stop=(k == 7))
                for hl in range(4):
                    ht = htile * 4 + hl
                    src = pst[0:HT_M[ht], hl * 512:(hl + 1) * 512]
                    bsl = slice(bc * 512, (bc + 1) * 512)
                    if ht < 4:
                        nc.scalar.copy(h0_sb[:, ht, bsl], src)
                    elif ht < 6:
                        nc.scalar.copy(h1_sb[:, ht - 4, bsl], src)
                    elif ht == 6:
                        nc.vector.tensor_copy(h2_sb[:, bsl], src)
                    else:
                        nc.vector.tensor_copy(h3_sb[0:64, bsl], src)

        # ---- main loop: logits -> exp (+Z accumulate) -> gather/ln --------
        # groups: (concat_off, width, n_ktiles, fp8, lhsT source, rhs source)
        KW = 2 if FP8_ON else 1

        def lh_head(kp, rt):
            return xt_sb[:, KW * kp:KW * kp + KW, rt * 128:(rt + 1) * 128]

        def lh_c0(kp, rt):
            return h0_sb[:, KW * kp:KW * kp + KW, rt * 128:(rt + 1) * 128]

        def lh_c1(kp, rt):
            return h1_sb[:, KW * kp:KW * kp + KW, rt * 128:(rt + 1) * 128]

        def lh_c2(kp, rt):
            return h2_sb[:, rt * 128:(rt + 1) * 128]

        def lh_c3(kp, rt):
            return h3_sb[0:64, rt * 128:(rt + 1) * 128]

        def rh_head(kp, a, w):
            return whead_sb[:, KW * kp:KW * kp + KW, a:a + w]

        def rh_w0(kp, a, w):
            return wout0_sb[:, KW * kp:KW * kp + KW, a:a + w]

        def rh_w1(kp, a, w):
            return wout1_sb[:, KW * kp:KW * kp + KW, a:a + w]

        def rh_w2(kp, a, w):
            return wout2_sb[:, a:a + w]

        def rh_w3(kp, a, w):
            return wout3_sb[0:64, a:a + w]

        E_HEAD = 1.0 / S_WHEAD
        E_C01 = 1.0 / (S_PROJ * S_WOUT)
        E_C23 = 1.0 / S_PROJ
        if FP8_ON:
            GROUPS = [
                (0, 1254, 4, True, lh_head, rh_head),
                (OFF_CL[0], 1250, 2, True, lh_c0, rh_w0),
                (OFF_CL[1], 2500, 1, True, lh_c1, rh_w1),
                (OFF_CL[2], 5000, 1, False, lh_c2, rh_w2),
                (OFF_CL[3], 2500, 1, False, lh_c3, rh_w3),
            ]
        else:
            GROUPS = [
                (0, 1254, 8, False, lh_head, rh_head),
                (OFF_CL[0], 1250, 4, False, lh_c0, rh_w0),
                (OFF_CL[1], 2500, 2, False, lh_c1, rh_w1),
                (OFF_CL[2], 5000, 1, False, lh_c2, rh_w2),
                (OFF_CL[3], 2500, 1, False, lh_c3, rh_w3),
            ]

        # PSUM piece bounds and the exp segments inside each piece
        PB = [0, 2048, 4096, 6144, 8192, 10240, 12288, CONCAT]
        # (lo, hi, scale, zslot or None); head excludes links [1250,1254)
        SEGS = [
            [(0, 1254, E_HEAD, None), (1254, 2048, E_C01, None)],
            [(2048, 2504, E_C01, None), (2504, 4096, E_C01, 0)],
            [(4096, 5004, E_C01, 1), (5004, 6144, E_C23, 2)],
            [(6144, 8192, E_C23, 3)],
            [(8192, 10004, E_C23, 4), (10004, 10240, E_C23, 5)],
            [(10240, 12288, E_C23, 6)],
            [(12288, CONCAT, E_C23, 7)],
        ]

        ENDLN = bool(os.environ.get("ADAK_ENDLN"))

        def emit_numer(rt):
            # ln of gathered exp values + fused weighted-sum numerator
            nc.scalar.activation(logv3[:, rt, :], vg3[:, rt, :], LN)
            nc.vector.tensor_tensor_reduce(
                tmpS[:, :], logv3[:, rt, :], wm_sb[:, rt, :], 1.0, 0.0,
                op0=MULT, op1=ADD, accum_out=pview[:, 10, rt:rt + 1])

        for rt in range(RT):
            expb = big.tile([128, CONCAT_PAD], BF16_DT, tag="big",
                            name=f"expb_{rt}")
            for pi in range(7):
                lo, hi = PB[pi], PB[pi + 1]
                pst = ps.tile([128, hi - lo], F32, tag="ps",
                              name=f"ps_{rt}_{pi}")
                for goff, width, kt, fp8, lh, rh in GROUPS:
                    slo, shi = max(goff, lo), min(goff + width, hi)
                    if slo >= shi:
                        continue
                    # chunks must not cross PSUM bank boundaries (512 f32,
                    # bank grid is piece-relative and pieces are 2048-aligned)
                    subs = []
                    a = slo
                    while a < shi:
                        w = min(shi - a, 512 - ((a - lo) % 512))
                        subs.append((a, w))
                        a += w
                    for kp in range(kt):
                        for a, w in subs:
                            nc.tensor.matmul(
                                pst[:, a - lo:a - lo + w],
                                lh(kp, rt), rh(kp, a - goff, w),
                                start=(kp == 0), stop=(kp == kt - 1),
                                perf_mode=DR if fp8 else None)
                for slo, shi, sc, zslot in SEGS[pi]:
                    acc = (zs[:, rt, zslot:zslot + 1]
                           if zslot is not None else None)
                    nc.scalar.activation(
                        expb[:, slo:shi], pst[:, slo - lo:shi - lo], EXP,
                        scale=sc, accum_out=acc)
                if pi == 0:
                    # raw link logits out of PSUM (their ln IS the logit)
                    nc.vector.tensor_scalar(
                        llinkraw[:, rt, :], pst[:, 1250 - lo:1254 - lo],
                        E_HEAD, None, op0=MULT)
                if pi == 3 and rt > 0 and not ENDLN:
                    emit_numer(rt - 1)
            # head/c0 Z partial sums on DVE (head must exclude link columns)
            for q, (za, zw) in enumerate(
                    [(0, SH_SHARD), (OFF_CL[0], CL_SHARD[0])]):
                nc.vector.tensor_scalar(
                    zscr[:, 0:zw],
                    expb[:, za:za + zw], 1.0, 0.0, op0=MULT, op1=ADD,
                    accum_out=pview[:, q, rt:rt + 1])
            # gather exp(logit) at this core's targets
            nc.gpsimd.indirect_copy(
                vg3[:, rt, :], expb[:, 0:CONCAT],
                tix_sb[:, rt * SW:(rt + 1) * SW], True)
            # per-row-tile statistics that don't need log(v): den, W_g
            # (Wsl is never needed: den already equals Wsl + sum_g Wg)
            nc.vector.tensor_reduce(
                pview[:, 5, rt:rt + 1], wm_sb[:, rt, :], AXX, ADD)
            for gi in range(1, 5):
                nc.vector.scalar_tensor_tensor(
                    tmpS[:, :], tgid_sb[:, rt, :], float(gi + 1), wm_sb[:, rt, :],
                    op0=ISEQ, op1=MULT)
                nc.vector.tensor_reduce(
                    pview[:, 5 + gi, rt:rt + 1], tmpS[:, :], AXX, ADD)
        if ENDLN:
            nc.scalar.activation(
                logv3[:, :, :].rearrange("p a b -> p (a b)"),
                vg3[:, :, :].rearrange("p a b -> p (a b)"), LN)
            tmp3 = vg3
            nc.vector.tensor_tensor(
                tmp3[:, :, :], logv3[:, :, :], wm_sb[:, :, :], MULT)
            nc.vector.tensor_reduce(pview[:, 10, :], tmp3[:, :, :], AXX, ADD)
        else:
            emit_numer(RT - 1)

        # combine the c1/c2/c3 Z piece-partials into the payload
        t8z = sb.tile([128, 8], F32)
        nc.vector.tensor_tensor(pview[:, 2, :], zs[:, :, 0], zs[:, :, 1], ADD)
        nc.vector.tensor_tensor(t8z[:, :], zs[:, :, 2], zs[:, :, 3], ADD)
        nc.vector.tensor_tensor(pview[:, 3, :], t8z[:, :], zs[:, :, 4], ADD)
        nc.vector.tensor_tensor(t8z[:, :], zs[:, :, 5], zs[:, :, 6], ADD)
        nc.vector.tensor_tensor(pview[:, 4, :], t8z[:, :], zs[:, :, 7], ADD)

        # ---- AllReduce the statistics ----
        cc_in = dr.tile([128, 88], F32)
        cc_out = dr.tile([128, 88], F32, addr_space="Shared")
        nc.sync.dma_start(out=cc_in, in_=pay[:, :])
        nc.gpsimd.collective_compute(
            "AllReduce", ADD,
            replica_groups=[list(range(NCORES))],
            ins=[cc_in.opt()], outs=[cc_out.opt()])
        nc.sync.dma_start(out=rsb, in_=cc_out)

        # ---- final combine (identical on every core) ----
        linkexp = sb.tile([128, 32], F32)
        lsum = sb.tile([128, 8], F32)
        zcomb = sb.tile([128, 40], F32)
        lnz = sb.tile([128, 40], F32)
        s8 = sb.tile([128, 8], F32)
        tA = sb.tile([128, 8], F32)
        num8 = sb.tile([128, 8], F32)
        rden = sb.tile([128, 8], F32)
        pcol = sb.tile([128, 1], F32)
        llview = llinkraw[:, :, :]

        nc.scalar.activation(
            linkexp[:, :], llview.rearrange("p a b -> p (a b)"), EXP)
        lexp3 = linkexp[:, :].rearrange("p (r g) -> p r g", g=4)
        nc.vector.tensor_reduce(lsum[:, :], lexp3, AXX, ADD)
        nc.vector.tensor_tensor(zcomb[:, 0:8], rview[:, 0, :], lsum[:, :], ADD)
        nc.vector.tensor_copy(zcomb[:, 8:40], rsb[:, 8:40])
        nc.scalar.activation(lnz[:, :], zcomb[:, :], LN)
        llink3 = llview.rearrange("p r g -> p g r")
        for g in range(4):
            nc.vector.tensor_tensor(
                tA[:, :], llink3[:, g, :], lnz[:, 8 + 8 * g:16 + 8 * g], SUB)
            if g == 0:
                nc.vector.tensor_tensor(s8[:, :], tA[:, :], rview[:, 6 + g, :], MULT)
            else:
                nc.vector.tensor_tensor(tA[:, :], tA[:, :], rview[:, 6 + g, :], MULT)
                nc.vector.tensor_tensor(s8[:, :], s8[:, :], tA[:, :], ADD)
        # num = numraw + s8 - den * logZh
        nc.vector.tensor_tensor(tA[:, :], rview[:, 5, :], lnz[:, 0:8], MULT)
        nc.vector.tensor_tensor(num8[:, :], rview[:, 10, :], tA[:, :], SUB)
        nc.vector.tensor_tensor(num8[:, :], num8[:, :], s8[:, :], ADD)
        nc.vector.reciprocal(rden[:, :], rview[:, 5, :])
        nc.vector.tensor_tensor(num8[:, :], num8[:, :], rden[:, :], MULT)
        nc.vector.tensor_reduce(pcol[:, :], num8[:, :], AXX, ADD)
        psq = ps.tile([1, 1], F32, tag="ps")
        nc.tensor.matmul(psq[0:1, 0:1], pcol[:, 0:1], ones_sb[:, 0:1],
                         start=True, stop=True)
        nc.scalar.mul(out_sb[:, :], psq[0:1, 0:1], -1.0 / (B + 1e-5))
        nc.sync.dma_start(out=out_d[:], in_=out_sb)

    nc.compile()
    _CACHE[S] = nc
    return nc


# ----------------------------------------------------------------------------
# host-side sharding / index routing
# ----------------------------------------------------------------------------


def _f8(a, scale):
    return np.clip(np.asarray(a, np.float32) * scale, -239.0, 239.0).astype(F8)


def _shard_inputs(features, head_weight, projs, outs, discard_probs,
                  targets, target_mask):
    """Build the 8 per-core input maps. Returns (in_maps, S)."""
    if FP8_ON:
        xt = _f8(np.ascontiguousarray(features.T), 1.0)
        projt = _f8(np.concatenate([p.T for p in projs], axis=1), S_PROJ)
    else:
        xt = np.ascontiguousarray(features.T).astype(BF16)
        projt = (np.concatenate([p.T for p in projs], axis=1)
                 * S_PROJ).astype(BF16)

    tgt = np.asarray(targets).astype(np.int64).reshape(-1)
    msk = np.asarray(target_mask).astype(bool).reshape(-1)
    bb = np.repeat(np.arange(B, dtype=np.int64), T)

    grp = np.digitize(tgt, GRP_BOUNDS[1:-1])          # 0..4 (0 = shortlist)
    u = tgt - np.asarray(GRP_BOUNDS)[grp]
    shard = np.asarray(GRP_SHARD)[grp]
    core = u // shard
    jcat = u % shard + np.asarray(GRP_OFF)[grp]
    wval = (1.0 - discard_probs[tgt]).astype(np.float32)

    rt = bb >> 7
    gc = (bb >> 4) & 7

    # padded slots per (core, rt, gc)
    key_all = ((core * RT + rt) * 8 + gc).astype(np.int64)
    valid = msk
    counts = np.bincount(key_all[valid], minlength=NCORES * RT * 8)
    # multiple of 32 so each row-tile's wrapped idx slice stays 4B-aligned
    S = int(counts.max())
    S = ((S + 31) // 32) * 32

    in_maps = []
    for c in range(NCORES):
        sel = valid & (core == c)
        jj = jcat[sel]
        bsel = bb[sel]
        gsel = grp[sel]
        rts = rt[sel]
        gcs = gc[sel]
        ww = wval[sel]
        po = bsel & 15
        key = rts * 8 + gcs
        order = np.argsort(key, kind="stable")
        jj, bsel, gsel, rts, gcs, po, ww = (a[order] for a in
                                            (jj, bsel, gsel, rts, gcs, po, ww))
        key = key[order]
        # slot within each (rt, gc) bucket
        start_of = np.r_[0, np.flatnonzero(np.diff(key)) + 1]
        bucket_len = np.diff(np.r_[start_of, len(key)])
        slot = np.arange(len(key)) - np.repeat(start_of, bucket_len)

        tix = np.full((128, RT * (S // 16)), PADIDX, np.uint16)
        tix[16 * gcs + slot % 16, rts * (S // 16) + slot // 16] = jj.astype(np.uint16)
        tgid = np.zeros((128, RT, S), np.float32)
        tgid[16 * gcs + po, rts, slot] = gsel + 1.0
        tgid = tgid.astype(BF16)
        wm = np.zeros((128, RT, S), np.float32)
        wm[16 * gcs + po, rts, slot] = ww
        wm = wm.astype(BF16)

        # head shard + link columns, transposed
        hslice = head_weight[c * SH_SHARD:(c + 1) * SH_SHARD]
        wh_cat = np.concatenate(
            [hslice.T, head_weight[SHORT:SHORT + 4].T], axis=1)
        wheadt = (_f8(wh_cat, S_WHEAD) if FP8_ON
                  else (wh_cat * S_WHEAD).astype(BF16))
        in_maps.append({
            "xt": xt,
            "projt": projt,
            "wheadt": wheadt,
            "wout0t": (_f8(outs[0][c * CL_SHARD[0]:(c + 1) * CL_SHARD[0]].T,
                           S_WOUT) if FP8_ON else
                       (outs[0][c * CL_SHARD[0]:(c + 1) * CL_SHARD[0]].T
                        * S_WOUT).astype(BF16)),
            "wout1t": (_f8(outs[1][c * CL_SHARD[1]:(c + 1) * CL_SHARD[1]].T,
                           S_WOUT) if FP8_ON else
                       (outs[1][c * CL_SHARD[1]:(c + 1) * CL_SHARD[1]].T
                        * S_WOUT).astype(BF16)),
            "wout2t": np.ascontiguousarray(
                outs[2][c * CL_SHARD[2]:(c + 1) * CL_SHARD[2]].T).astype(BF16),
            "wout3t": np.ascontiguousarray(
                outs[3][c * CL_SHARD[3]:(c + 1) * CL_SHARD[3]].T).astype(BF16),
            "tgtidx": tix,
            "tgid": tgid,
            "wm": wm,
        })
    return in_maps, S


def _run(features, head_weight, proj0, out0, proj1, out1, proj2, out2,
         proj3, out3, discard_probs, targets, target_mask,
         trace=False, tmpdir=None):
    features = np.asarray(features, np.float32)
    head_weight = np.asarray(head_weight, np.float32)
    projs = [np.asarray(p, np.float32) for p in (proj0, proj1, proj2, proj3)]
    outs = [np.asarray(o, np.float32) for o in (out0, out1, out2, out3)]
    discard_probs = np.asarray(discard_probs, np.float32)

    in_maps, S = _shard_inputs(features, head_weight, projs, outs,
                               discard_probs, targets, target_mask)
    nc = _build(S)
    res = run_bass_kernel_spmd(nc, in_maps, list(range(NCORES)),
                               trace=trace, tmpdir=tmpdir)
    val = np.asarray(res.results[0]["out"], np.float32).reshape(())
    return val, res


def kernel(**inputs) -> np.ndarray:
    val, _ = _run(**inputs)
    return val


# revision 10
# speedup vs baseline: 1.0565x; 1.0565x over previous
"""Adaptive-softmax loss (nn_AdaptiveLoss) on 8 trn2 NeuronCores.

Strategy: tensor-parallel over the vocab dimension, 8-way. Each core owns
1/8 of the shortlist head columns and 1/8 of each tail cluster's output
rows. Per core:

  - computes cluster hidden states h_g = x @ proj_g.T (replicated, small)
    with fp8 DoubleRow matmuls; h0/h1 re-quantized to fp8, h2/h3 to bf16,
  - computes its slice of every group's logits (fp8 DoubleRow where the
    contraction depth allows pairing, bf16 for the K<=128 clusters),
  - exp()s the logits on ACT with per-group scale folding the fp8
    scale factors; softmax partials Z_g[b] ride the ACT accumulator
    (clusters) and DVE tensor_scalar accumulators (head/c0),
  - gathers exp(logit) at this core's share of the targets straight out
    of SBUF (gpsimd indirect_copy), takes ln in-loop (the exp+ln combined
    activation table is preloaded so no table thrash),
  - reduces per-row partial loss statistics (numerator, weight sums, Z),
  - one 45KB AllReduce combines the statistics, then every core finishes
    the (cheap) log/normalize arithmetic identically and writes the scalar.

The full [B, VOCAB] log-prob matrix is never materialized anywhere.
"""

import sys

sys.path.insert(0, "/opt/trn_rl_repo")

from contextlib import ExitStack

import ml_dtypes
import numpy as np

import concourse.bass as bass  # noqa: F401  (engine types via nc.*)
import concourse.mybir as mybir
import concourse.tile as tile
from concourse import bacc
from concourse.bass_utils import run_bass_kernel_spmd

BF16 = ml_dtypes.bfloat16
F8 = ml_dtypes.float8_e4m3
F32 = mybir.dt.float32
BF16_DT = mybir.dt.bfloat16
F8_DT = mybir.dt.float8e4
U16 = mybir.dt.uint16

NCORES = 8
B, T, D = 1024, 128, 1024
VOCAB, SHORT = 100000, 10000
CL_SIZES = [10000, 20000, 40000, 20000]
CL_D = [512, 256, 128, 64]
SH_SHARD = SHORT // NCORES                      # 1250
CL_SHARD = [s // NCORES for s in CL_SIZES]      # 1250 2500 5000 2500
GRP_BOUNDS = [0, 10000, 20000, 40000, 80000, 100000]
GRP_SHARD = [SH_SHARD] + CL_SHARD

# per-core concatenated logits layout: [head | links(4) | c0 | c1 | c2 | c3]
OFF_HEAD = 0
OFF_LINK = SH_SHARD                              # 1250
OFF_CL = [1254, 2504, 5004, 10004]
GRP_OFF = [OFF_HEAD] + OFF_CL                    # per-group concat offset
CONCAT = OFF_CL[-1] + CL_SHARD[-1]               # 12504
CONCAT_PAD = 12544
# pad slots gather column 0 (always computed, finite); their tgid==0 makes
# the ownership mask zero the contribution.
PADIDX = 0
RT = 8                                           # row tiles of 128

# fp8 scale factors (folded back out via the exp activation scale)
S_WHEAD = 64.0                                   # head weight prescale
S_PROJ = 32.0                                    # proj prescale -> h scale
S_WOUT = 64.0                                    # c0/c1 out-proj prescale
EXP_TABLE_ID = 6                                 # natural_log_exp_and_others
import os as _os
FP8_ON = not _os.environ.get("ADAK_BF16")


# ----------------------------------------------------------------------------
# device kernel builder
# ----------------------------------------------------------------------------

_CACHE: dict[int, object] = {}


def _build(S: int):
    """Build + compile the SPMD kernel for padded slot count S (multiple of 16)."""
    if S in _CACHE:
        return _CACHE[S]
    SW = S // 16

    nc = bacc.Bacc("TRN2", target_bir_lowering=False, debug=False,
                   num_devices=NCORES)

    MMDT = F8_DT if FP8_ON else BF16_DT
    xt_d = nc.dram_tensor("xt", [D, B], MMDT, kind="ExternalInput")
    projt_d = nc.dram_tensor("projt", [D, sum(CL_D)], MMDT, kind="ExternalInput")
    whead_d = nc.dram_tensor("wheadt", [D, 1254], MMDT, kind="ExternalInput")
    wout0_d = nc.dram_tensor("wout0t", [CL_D[0], CL_SHARD[0]], MMDT,
                             kind="ExternalInput")
    wout1_d = nc.dram_tensor("wout1t", [CL_D[1], CL_SHARD[1]], MMDT,
                             kind="ExternalInput")
    wout2_d = nc.dram_tensor("wout2t", [CL_D[2], CL_SHARD[2]], BF16_DT,
                             kind="ExternalInput")
    wout3_d = nc.dram_tensor("wout3t", [CL_D[3], CL_SHARD[3]], BF16_DT,
                             kind="ExternalInput")
    tix_d = nc.dram_tensor("tgtidx", [128, RT * SW], U16, kind="ExternalInput")
    tgid_d = nc.dram_tensor("tgid", [128, RT, S], BF16_DT, kind="ExternalInput")
    wm_d = nc.dram_tensor("wm", [128, RT, S], BF16_DT, kind="ExternalInput")
    out_d = nc.dram_tensor("out", [1, 1], F32, kind="ExternalOutput")

    EXP = mybir.ActivationFunctionType.Exp
    LN = mybir.ActivationFunctionType.Ln
    ADD = mybir.AluOpType.add
    SUB = mybir.AluOpType.subtract
    MULT = mybir.AluOpType.mult
    ISEQ = mybir.AluOpType.is_equal
    AXX = mybir.AxisListType.X
    DR = mybir.MatmulPerfMode.DoubleRow

    with tile.TileContext(nc) as tc, ExitStack() as ctx:
        sb = ctx.enter_context(tc.tile_pool(name="sb", bufs=1))
        big = ctx.enter_context(tc.tile_pool(name="big", bufs=3))
        ps = ctx.enter_context(tc.tile_pool(name="ps", bufs=2, space="PSUM"))
        dr = ctx.enter_context(tc.tile_pool(name="dr", bufs=1, space="DRAM"))

        # combined exp+ln activation table so EXP and LN interleave with a
        # single table load for the whole kernel
        import os
        if not os.environ.get("ADAK_NO_TABLE_PRELOAD"):
            nc.scalar.add_instruction(mybir.InstLoadActFuncSet(
                name=nc.get_next_instruction_name(),
                act_func_set_id=EXP_TABLE_ID, ins=[], outs=[]))

        # ---- persistent SBUF tensors ----
        xt_sb = sb.tile([128, 8, B], MMDT)             # x.T  [d, b] k-tiled
        whead_sb = sb.tile([128, 8, 1254], MMDT)
        wout0_sb = sb.tile([128, 4, CL_SHARD[0]], MMDT)
        wout1_sb = sb.tile([128, 2, CL_SHARD[1]], MMDT)
        wout2_sb = sb.tile([128, CL_SHARD[2]], BF16_DT)
        wout3_sb = sb.tile([64, CL_SHARD[3]], BF16_DT)
        h0_sb = sb.tile([128, 4, B], MMDT)             # h.T (x S_PROJ)
        h1_sb = sb.tile([128, 2, B], MMDT)
        h2_sb = sb.tile([128, B], BF16_DT)
        h3_sb = sb.tile([64, B], BF16_DT)
        tmpS = sb.tile([128, S], BF16_DT)
        tix_sb = sb.tile([128, RT * SW], U16)
        tgid_sb = sb.tile([128, RT, S], BF16_DT)
        vg3 = sb.tile([128, RT, S], BF16_DT)           # gathered exp(logit)
        wm_sb = sb.tile([128, RT, S], BF16_DT)         # (1-dp)*ownership
        logv3 = sb.tile([128, RT, S], BF16_DT)
        llinkraw = sb.tile([128, RT, 4], F32)          # raw link logits
        zscr = sb.tile([128, 1280], BF16_DT)
        zs = sb.tile([128, RT, 8], F32)   # Z partials per piece, c1/c2/c3
        # AR payload, 11 stats x 8 row-tiles:
        # q: 0 Zh, 1..4 Zc_g, 5 den, 6..9 Wg, 10 numraw
        pay = sb.tile([128, 88], F32)
        rsb = sb.tile([128, 88], F32)
        ones_sb = sb.tile([128, 1], F32)
        out_sb = sb.tile([1, 1], F32)

        pview = pay[:, :].rearrange("p (q r) -> p q r", q=11)
        rview = rsb[:, :].rearrange("p (q r) -> p q r", q=11)

        # ---- input DMAs (order matters: compute-critical tensors first;
        # xt/projt interleaved per k-tile so the h matmuls start early) ----
        pj = sb.tile([128, 8, sum(CL_D)], MMDT)
        xt_r = xt_d.ap().rearrange("(k p) b -> p k b", p=128)
        pj_r = projt_d.ap().rearrange("(k p) c -> p k c", p=128)
        for k in range(8):
            nc.sync.dma_start(out=xt_sb[:, k, :], in_=xt_r[:, k, :])
            nc.sync.dma_start(out=pj[:, k, :], in_=pj_r[:, k, :])
        nc.sync.dma_start(out=whead_sb,
                          in_=whead_d.ap().rearrange("(k p) c -> p k c", p=128))
        nc.sync.dma_start(out=wout0_sb,
                          in_=wout0_d.ap().rearrange("(k p) c -> p k c", p=128))
        nc.sync.dma_start(out=wout1_sb,
                          in_=wout1_d.ap().rearrange("(k p) c -> p k c", p=128))
        nc.sync.dma_start(out=wout2_sb, in_=wout2_d[:])
        nc.sync.dma_start(out=wout3_sb, in_=wout3_d[:])
        nc.sync.dma_start(out=tix_sb, in_=tix_d[:])
        nc.sync.dma_start(out=tgid_sb, in_=tgid_d[:])
        nc.sync.dma_start(out=wm_sb, in_=wm_d[:])

        nc.vector.memset(ones_sb[:, :], 1.0)

        # Prewarm the collectives path: a tiny dummy AllReduce early in the
        # run (nothing in the loop depends on it) absorbs the ~11.5us
        # first-collective ncfw setup and part of the cross-core launch skew,
        # so the real AllReduce at the tail starts almost immediately.
        warm_src = sb.tile([1, 16], F32)
        warm_sb = sb.tile([1, 16], F32)
        warm_in = dr.tile([1, 16], F32)
        warm_out = dr.tile([1, 16], F32, addr_space="Shared")
        nc.vector.memset(warm_src[:, :], 1.0)
        nc.sync.dma_start(out=warm_in, in_=warm_src[:, :])
        nc.gpsimd.collective_compute(
            "AllReduce", ADD, replica_groups=[list(range(NCORES))],
            ins=[warm_in.opt()], outs=[warm_out.opt()])
        nc.sync.dma_start(out=warm_sb, in_=warm_out)
        nc.vector.tensor_copy(zscr[0:1, 0:16], warm_sb[:, :])

        # ---- cluster hidden states h.T (all batch rows, computed locally —
        # a sharded h + AllGather is infeasible: the collectives entry
        # barrier isn't done until ~60-150us into the NEFF) ----
        HT_OFF = [0, 128, 256, 384, 512, 640, 768, 896]
        HT_M = [128, 128, 128, 128, 128, 128, 128, 64]
        for bc in range(2):
            for htile in range(2):
                pst = ps.tile([128, 2048], F32, tag="ps", name=f"hps_{bc}_{htile}")
                for hl in range(4):
                    ht = htile * 4 + hl
                    M = HT_M[ht]
                    if FP8_ON:
                        for kp in range(4):
                            nc.tensor.matmul(
                                pst[0:M, hl * 512:(hl + 1) * 512],
                                pj[:, 2 * kp:2 * kp + 2, HT_OFF[ht]:HT_OFF[ht] + M],
                                xt_sb[:, 2 * kp:2 * kp + 2, bc * 512:(bc + 1) * 512],
                                start=(kp == 0), stop=(kp == 3), perf_mode=DR)
                    else:
                        for k in range(8):
                            nc.tensor.matmul(
                                pst[0:M, hl * 512:(hl + 1) * 512],
                                pj[:, k, HT_OFF[ht]:HT_OFF[ht] + M],
                                xt_sb[:, k, bc * 512:(bc + 1) * 512],
                                start=(k == 0), stop=(k == 7))
                for hl in range(4):
                    ht = htile * 4 + hl
                    src = pst[0:HT_M[ht], hl * 512:(hl + 1) * 512]
                    bsl = slice(bc * 512, (bc + 1) * 512)
                    if ht < 4:
                        nc.scalar.copy(h0_sb[:, ht, bsl], src)
                    elif ht < 6:
                        nc.scalar.copy(h1_sb[:, ht - 4, bsl], src)
                    elif ht == 6:
                        nc.vector.tensor_copy(h2_sb[:, bsl], src)
                    else:
                        nc.vector.tensor_copy(h3_sb[0:64, bsl], src)

        # ---- main loop: logits -> exp (+Z accumulate) -> gather/ln --------
        # groups: (concat_off, width, n_ktiles, fp8, lhsT source, rhs source)
        KW = 2 if FP8_ON else 1

        def lh_head(kp, rt):
            return xt_sb[:, KW * kp:KW * kp + KW, rt * 128:(rt + 1) * 128]

        def lh_c0(kp, rt):
            return h0_sb[:, KW * kp:KW * kp + KW, rt * 128:(rt + 1) * 128]

        def lh_c1(kp, rt):
            return h1_sb[:, KW * kp:KW * kp + KW, rt * 128:(rt + 1) * 128]

        def lh_c2(kp, rt):
            return h2_sb[:, rt * 128:(rt + 1) * 128]

        def lh_c3(kp, rt):
            return h3_sb[0:64, rt * 128:(rt + 1) * 128]

        def rh_head(kp, a, w):
            return whead_sb[:, KW * kp:KW * kp + KW, a:a + w]

        def rh_w0(kp, a, w):
            return wout0_sb[:, KW * kp:KW * kp + KW, a:a + w]

        def rh_w1(kp, a, w):
            return wout1_sb[:, KW * kp:KW * kp + KW, a:a + w]

        def rh_w2(kp, a, w):
            return wout2_sb[:, a:a + w]

        def rh_w3(kp, a, w):
            return wout3_sb[0:64, a:a + w]

        E_HEAD = 1.0 / S_WHEAD
        E_C01 = 1.0 / (S_PROJ * S_WOUT)
        E_C23 = 1.0 / S_PROJ
        if FP8_ON:
            GROUPS = [
                (0, 1254, 4, True, lh_head, rh_head),
                (OFF_CL[0], 1250, 2, True, lh_c0, rh_w0),
                (OFF_CL[1], 2500, 1, True, lh_c1, rh_w1),
                (OFF_CL[2], 5000, 1, False, lh_c2, rh_w2),
                (OFF_CL[3], 2500, 1, False, lh_c3, rh_w3),
            ]
        else:
            GROUPS = [
                (0, 1254, 8, False, lh_head, rh_head),
                (OFF_CL[0], 1250, 4, False, lh_c0, rh_w0),
                (OFF_CL[1], 2500, 2, False, lh_c1, rh_w1),
                (OFF_CL[2], 5000, 1, False, lh_c2, rh_w2),
                (OFF_CL[3], 2500, 1, False, lh_c3, rh_w3),
            ]

        # PSUM piece bounds and the exp segments inside each piece
        PB = [0, 2048, 4096, 6144, 8192, 10240, 12288, CONCAT]
        # (lo, hi, scale, zslot or None); head excludes links [1250,1254)
        SEGS = [
            [(0, 1254, E_HEAD, None), (1254, 2048, E_C01, None)],
            [(2048, 2504, E_C01, None), (2504, 4096, E_C01, 0)],
            [(4096, 5004, E_C01, 1), (5004, 6144, E_C23, 2)],
            [(6144, 8192, E_C23, 3)],
            [(8192, 10004, E_C23, 4), (10004, 10240, E_C23, 5)],
            [(10240, 12288, E_C23, 6)],
            [(12288, CONCAT, E_C23, 7)],
        ]

        ENDLN = bool(os.environ.get("ADAK_ENDLN"))

        def emit_numer(rt):
            # ln of gathered exp values + weighted-sum numerator
            # (tensor_tensor_reduce faults at runtime on hw — avoid)
            nc.scalar.activation(logv3[:, rt, :], vg3[:, rt, :], LN)
            nc.vector.tensor_tensor(
                tmpS[:, :], logv3[:, rt, :], wm_sb[:, rt, :], MULT)
            nc.vector.tensor_reduce(
                pview[:, 10, rt:rt + 1], tmpS[:, :], AXX, ADD)

        for rt in range(RT):
            expb = big.tile([128, CONCAT_PAD], BF16_DT, tag="big",
                            name=f"expb_{rt}")
            for pi in range(7):
                lo, hi = PB[pi], PB[pi + 1]
                pst = ps.tile([128, hi - lo], F32, tag="ps",
                              name=f"ps_{rt}_{pi}")
                for goff, width, kt, fp8, lh, rh in GROUPS:
                    slo, shi = max(goff, lo), min(goff + width, hi)
                    if slo >= shi:
                        continue
                    # chunks must not cross PSUM bank boundaries (512 f32,
                    # bank grid is piece-relative and pieces are 2048-aligned)
                    subs = []
                    a = slo
                    while a < shi:
                        w = min(shi - a, 512 - ((a - lo) % 512))
                        subs.append((a, w))
                        a += w
                    for kp in range(kt):
                        for a, w in subs:
                            nc.tensor.matmul(
                                pst[:, a - lo:a - lo + w],
                                lh(kp, rt), rh(kp, a - goff, w),
                                start=(kp == 0), stop=(kp == kt - 1),
                                perf_mode=DR if fp8 else None)
                for slo, shi, sc, zslot in SEGS[pi]:
                    acc = (zs[:, rt, zslot:zslot + 1]
                           if zslot is not None else None)
                    nc.scalar.activation(
                        expb[:, slo:shi], pst[:, slo - lo:shi - lo], EXP,
                        scale=sc, accum_out=acc)
                if pi == 0:
                    # raw link logits out of PSUM (their ln IS the logit)
                    nc.vector.tensor_scalar(
                        llinkraw[:, rt, :], pst[:, 1250 - lo:1254 - lo],
                        E_HEAD, None, op0=MULT)
                if pi == 3 and rt > 0 and not ENDLN:
                    emit_numer(rt - 1)
            # head/c0 Z partial sums on DVE (head must exclude link columns)
            for q, (za, zw) in enumerate(
                    [(0, SH_SHARD), (OFF_CL[0], CL_SHARD[0])]):
                nc.vector.tensor_scalar(
                    zscr[:, 0:zw],
                    expb[:, za:za + zw], 1.0, 0.0, op0=MULT, op1=ADD,
                    accum_out=pview[:, q, rt:rt + 1])
            # gather exp(logit) at this core's targets
            nc.gpsimd.indirect_copy(
                vg3[:, rt, :], expb[:, 0:CONCAT],
                tix_sb[:, rt * SW:(rt + 1) * SW], True)
            # per-row-tile statistics that don't need log(v): den, W_g
            # (Wsl is never needed: den already equals Wsl + sum_g Wg)
            nc.vector.tensor_reduce(
                pview[:, 5, rt:rt + 1], wm_sb[:, rt, :], AXX, ADD)
            for gi in range(1, 5):
                nc.vector.scalar_tensor_tensor(
                    tmpS[:, :], tgid_sb[:, rt, :], float(gi + 1), wm_sb[:, rt, :],
                    op0=ISEQ, op1=MULT)
                nc.vector.tensor_reduce(
                    pview[:, 5 + gi, rt:rt + 1], tmpS[:, :], AXX, ADD)
        if ENDLN:
            nc.scalar.activation(
                logv3[:, :, :].rearrange("p a b -> p (a b)"),
                vg3[:, :, :].rearrange("p a b -> p (a b)"), LN)
            tmp3 = vg3
            nc.vector.tensor_tensor(
                tmp3[:, :, :], logv3[:, :, :], wm_sb[:, :, :], MULT)
            nc.vector.tensor_reduce(pview[:, 10, :], tmp3[:, :, :], AXX, ADD)
        else:
            emit_numer(RT - 1)

        # combine the c1/c2/c3 Z piece-partials into the payload
        t8z = sb.tile([128, 8], F32)
        nc.vector.tensor_tensor(pview[:, 2, :], zs[:, :, 0], zs[:, :, 1], ADD)
        nc.vector.tensor_tensor(t8z[:, :], zs[:, :, 2], zs[:, :, 3], ADD)
        nc.vector.tensor_tensor(pview[:, 3, :], t8z[:, :], zs[:, :, 4], ADD)
        nc.vector.tensor_tensor(t8z[:, :], zs[:, :, 5], zs[:, :, 6], ADD)
        nc.vector.tensor_tensor(pview[:, 4, :], t8z[:, :], zs[:, :, 7], ADD)

        # ---- AllReduce the statistics ----
        cc_in = dr.tile([128, 88], F32)
        cc_out = dr.tile([128, 88], F32, addr_space="Shared")
        nc.sync.dma_start(out=cc_in, in_=pay[:, :])
        nc.gpsimd.collective_compute(
            "AllReduce", ADD,
            replica_groups=[list(range(NCORES))],
            ins=[cc_in.opt()], outs=[cc_out.opt()])
        nc.sync.dma_start(out=rsb, in_=cc_out)

        # ---- final combine (identical on every core) ----
        linkexp = sb.tile([128, 32], F32)
        lsum = sb.tile([128, 8], F32)
        zcomb = sb.tile([128, 40], F32)
        lnz = sb.tile([128, 40], F32)
        s8 = sb.tile([128, 8], F32)
        tA = sb.tile([128, 8], F32)
        num8 = sb.tile([128, 8], F32)
        rden = sb.tile([128, 8], F32)
        pcol = sb.tile([128, 1], F32)
        llview = llinkraw[:, :, :]

        nc.scalar.activation(
            linkexp[:, :], llview.rearrange("p a b -> p (a b)"), EXP)
        lexp3 = linkexp[:, :].rearrange("p (r g) -> p r g", g=4)
        nc.vector.tensor_reduce(lsum[:, :], lexp3, AXX, ADD)
        nc.vector.tensor_tensor(zcomb[:, 0:8], rview[:, 0, :], lsum[:, :], ADD)
        nc.vector.tensor_copy(zcomb[:, 8:40], rsb[:, 8:40])
        nc.scalar.activation(lnz[:, :], zcomb[:, :], LN)
        llink3 = llview.rearrange("p r g -> p g r")
        for g in range(4):
            nc.vector.tensor_tensor(
                tA[:, :], llink3[:, g, :], lnz[:, 8 + 8 * g:16 + 8 * g], SUB)
            if g == 0:
                nc.vector.tensor_tensor(s8[:, :], tA[:, :], rview[:, 6 + g, :], MULT)
            else:
                nc.vector.tensor_tensor(tA[:, :], tA[:, :], rview[:, 6 + g, :], MULT)
                nc.vector.tensor_tensor(s8[:, :], s8[:, :], tA[:, :], ADD)
        # num = numraw + s8 - den * logZh
        nc.vector.tensor_tensor(tA[:, :], rview[:, 5, :], lnz[:, 0:8], MULT)
        nc.vector.tensor_tensor(num8[:, :], rview[:, 10, :], tA[:, :], SUB)
        nc.vector.tensor_tensor(num8[:, :], num8[:, :], s8[:, :], ADD)
        nc.vector.reciprocal(rden[:, :], rview[:, 5, :])
        nc.vector.tensor_tensor(num8[:, :], num8[:, :], rden[:, :], MULT)
        nc.vector.tensor_reduce(pcol[:, :], num8[:, :], AXX, ADD)
        psq = ps.tile([1, 1], F32, tag="ps")
        nc.tensor.matmul(psq[0:1, 0:1], pcol[:, 0:1], ones_sb[:, 0:1],
                         start=True, stop=True)
        nc.scalar.mul(out_sb[:, :], psq[0:1, 0:1], -1.0 / (B + 1e-5))
        nc.sync.dma_start(out=out_d[:], in_=out_sb)

    nc.compile()
    _CACHE[S] = nc
    return nc


# ----------------------------------------------------------------------------
# host-side sharding / index routing
# ----------------------------------------------------------------------------


def _f8(a, scale):
    return np.clip(np.asarray(a, np.float32) * scale, -239.0, 239.0).astype(F8)


def _shard_inputs(features, head_weight, projs, outs, discard_probs,
                  targets, target_mask):
    """Build the 8 per-core input maps. Returns (in_maps, S)."""
    if FP8_ON:
        xt = _f8(np.ascontiguousarray(features.T), 1.0)
        projt = _f8(np.concatenate([p.T for p in projs], axis=1), S_PROJ)
    else:
        xt = np.ascontiguousarray(features.T).astype(BF16)
        projt = (np.concatenate([p.T for p in projs], axis=1)
                 * S_PROJ).astype(BF16)

    tgt = np.asarray(targets).astype(np.int64).reshape(-1)
    msk = np.asarray(target_mask).astype(bool).reshape(-1)
    bb = np.repeat(np.arange(B, dtype=np.int64), T)

    grp = np.digitize(tgt, GRP_BOUNDS[1:-1])          # 0..4 (0 = shortlist)
    u = tgt - np.asarray(GRP_BOUNDS)[grp]
    shard = np.asarray(GRP_SHARD)[grp]
    core = u // shard
    jcat = u % shard + np.asarray(GRP_OFF)[grp]
    wval = (1.0 - discard_probs[tgt]).astype(np.float32)

    rt = bb >> 7
    gc = (bb >> 4) & 7

    # padded slots per (core, rt, gc)
    key_all = ((core * RT + rt) * 8 + gc).astype(np.int64)
    valid = msk
    counts = np.bincount(key_all[valid], minlength=NCORES * RT * 8)
    # multiple of 32 so each row-tile's wrapped idx slice stays 4B-aligned
    S = int(counts.max())
    S = ((S + 31) // 32) * 32

    in_maps = []
    for c in range(NCORES):
        sel = valid & (core == c)
        jj = jcat[sel]
        bsel = bb[sel]
        gsel = grp[sel]
        rts = rt[sel]
        gcs = gc[sel]
        ww = wval[sel]
        po = bsel & 15
        key = rts * 8 + gcs
        order = np.argsort(key, kind="stable")
        jj, bsel, gsel, rts, gcs, po, ww = (a[order] for a in
                                            (jj, bsel, gsel, rts, gcs, po, ww))
        key = key[order]
        # slot within each (rt, gc) bucket
        start_of = np.r_[0, np.flatnonzero(np.diff(key)) + 1]
        bucket_len = np.diff(np.r_[start_of, len(key)])
        slot = np.arange(len(key)) - np.repeat(start_of, bucket_len)

        tix = np.full((128, RT * (S // 16)), PADIDX, np.uint16)
        tix[16 * gcs + slot % 16, rts * (S // 16) + slot // 16] = jj.astype(np.uint16)
        tgid = np.zeros((128, RT, S), np.float32)
        tgid[16 * gcs + po, rts, slot] = gsel + 1.0
        tgid = tgid.astype(BF16)
        wm = np.zeros((128, RT, S), np.float32)
        wm[16 * gcs + po, rts, slot] = ww
        wm = wm.astype(BF16)

        # head shard + link columns, transposed
        hslice = head_weight[c * SH_SHARD:(c + 1) * SH_SHARD]
        wh_cat = np.concatenate(
            [hslice.T, head_weight[SHORT:SHORT + 4].T], axis=1)
        wheadt = (_f8(wh_cat, S_WHEAD) if FP8_ON
                  else (wh_cat * S_WHEAD).astype(BF16))
        in_maps.append({
            "xt": xt,
            "projt": projt,
            "wheadt": wheadt,
            "wout0t": (_f8(outs[0][c * CL_SHARD[0]:(c + 1) * CL_SHARD[0]].T,
                           S_WOUT) if FP8_ON else
                       (outs[0][c * CL_SHARD[0]:(c + 1) * CL_SHARD[0]].T
                        * S_WOUT).astype(BF16)),
            "wout1t": (_f8(outs[1][c * CL_SHARD[1]:(c + 1) * CL_SHARD[1]].T,
                           S_WOUT) if FP8_ON else
                       (outs[1][c * CL_SHARD[1]:(c + 1) * CL_SHARD[1]].T
                        * S_WOUT).astype(BF16)),
            "wout2t": np.ascontiguousarray(
                outs[2][c * CL_SHARD[2]:(c + 1) * CL_SHARD[2]].T).astype(BF16),
            "wout3t": np.ascontiguousarray(
                outs[3][c * CL_SHARD[3]:(c + 1) * CL_SHARD[3]].T).astype(BF16),
            "tgtidx": tix,
            "tgid": tgid,
            "wm": wm,
        })
    return in_maps, S


def _run(features, head_weight, proj0, out0, proj1, out1, proj2, out2,
         proj3, out3, discard_probs, targets, target_mask,
         trace=False, tmpdir=None):
    features = np.asarray(features, np.float32)
    head_weight = np.asarray(head_weight, np.float32)
    projs = [np.asarray(p, np.float32) for p in (proj0, proj1, proj2, proj3)]
    outs = [np.asarray(o, np.float32) for o in (out0, out1, out2, out3)]
    discard_probs = np.asarray(discard_probs, np.float32)

    in_maps, S = _shard_inputs(features, head_weight, projs, outs,
                               discard_probs, targets, target_mask)
    nc = _build(S)
    res = run_bass_kernel_spmd(nc, in_maps, list(range(NCORES)),
                               trace=trace, tmpdir=tmpdir)
    val = np.asarray(res.results[0]["out"], np.float32).reshape(())
    return val, res


def kernel(**inputs) -> np.ndarray:
    val, _ = _run(**inputs)
    return val


# revision 11
# speedup vs baseline: 1.1319x; 1.0713x over previous
"""Adaptive-softmax loss (nn_AdaptiveLoss) on 8 trn2 NeuronCores.

Strategy: tensor-parallel over the vocab dimension, 8-way. Each core owns
1/8 of the shortlist head columns and 1/8 of each tail cluster's output
rows. Per core:

  - computes cluster hidden states h_g = x @ proj_g.T (replicated, small)
    with fp8 DoubleRow matmuls; h0/h1 re-quantized to fp8, h2/h3 to bf16,
  - computes its slice of every group's logits (fp8 DoubleRow where the
    contraction depth allows pairing, bf16 for the K<=128 clusters),
  - exp()s the logits on ACT with per-group scale folding the fp8
    scale factors; softmax partials Z_g[b] ride the ACT accumulator
    (clusters) and DVE tensor_scalar accumulators (head/c0),
  - gathers exp(logit) at this core's share of the targets straight out
    of SBUF (gpsimd indirect_copy), takes ln in-loop (the exp+ln combined
    activation table is preloaded so no table thrash),
  - reduces per-row partial loss statistics (numerator, weight sums, Z),
  - one 45KB AllReduce combines the statistics, then every core finishes
    the (cheap) log/normalize arithmetic identically and writes the scalar.

The full [B, VOCAB] log-prob matrix is never materialized anywhere.
"""

import sys

sys.path.insert(0, "/opt/trn_rl_repo")

from contextlib import ExitStack

import ml_dtypes
import numpy as np

import concourse.bass as bass  # noqa: F401  (engine types via nc.*)
import concourse.mybir as mybir
import concourse.tile as tile
from concourse import bacc
from concourse.bass_utils import run_bass_kernel_spmd

BF16 = ml_dtypes.bfloat16
F8 = ml_dtypes.float8_e4m3
F32 = mybir.dt.float32
BF16_DT = mybir.dt.bfloat16
F8_DT = mybir.dt.float8e4
U16 = mybir.dt.uint16

NCORES = 8
B, T, D = 1024, 128, 1024
VOCAB, SHORT = 100000, 10000
CL_SIZES = [10000, 20000, 40000, 20000]
CL_D = [512, 256, 128, 64]
SH_SHARD = SHORT // NCORES                      # 1250
CL_SHARD = [s // NCORES for s in CL_SIZES]      # 1250 2500 5000 2500
GRP_BOUNDS = [0, 10000, 20000, 40000, 80000, 100000]
GRP_SHARD = [SH_SHARD] + CL_SHARD

# per-core concatenated logits layout: [head | links(4) | c0 | c1 | c2 | c3]
OFF_HEAD = 0
OFF_LINK = SH_SHARD                              # 1250
OFF_CL = [1254, 2504, 5004, 10004]
GRP_OFF = [OFF_HEAD] + OFF_CL                    # per-group concat offset
CONCAT = OFF_CL[-1] + CL_SHARD[-1]               # 12504
CONCAT_PAD = 12544
# pad slots gather column 0 (always computed, finite); their tgid==0 makes
# the ownership mask zero the contribution.
PADIDX = 0
RT = 8                                           # row tiles of 128

# fp8 scale factors (folded back out via the exp activation scale)
S_WHEAD = 64.0                                   # head weight prescale
S_PROJ = 32.0                                    # proj prescale -> h scale
S_WOUT = 64.0                                    # c0/c1 out-proj prescale
EXP_TABLE_ID = 6                                 # natural_log_exp_and_others
import os as _os
FP8_ON = not _os.environ.get("ADAK_BF16")


# ----------------------------------------------------------------------------
# device kernel builder
# ----------------------------------------------------------------------------

_CACHE: dict[int, object] = {}


def _build(S: int):
    """Build + compile the SPMD kernel for padded slot count S (multiple of 16)."""
    if S in _CACHE:
        return _CACHE[S]
    SW = S // 16

    nc = bacc.Bacc("TRN2", target_bir_lowering=False, debug=False,
                   num_devices=NCORES)

    MMDT = F8_DT if FP8_ON else BF16_DT
    xt_d = nc.dram_tensor("xt", [D, B], MMDT, kind="ExternalInput")
    projt_d = nc.dram_tensor("projt", [D, sum(CL_D)], MMDT, kind="ExternalInput")
    whead_d = nc.dram_tensor("wheadt", [D, 1254], MMDT, kind="ExternalInput")
    wout0_d = nc.dram_tensor("wout0t", [CL_D[0], CL_SHARD[0]], MMDT,
                             kind="ExternalInput")
    wout1_d = nc.dram_tensor("wout1t", [CL_D[1], CL_SHARD[1]], MMDT,
                             kind="ExternalInput")
    wout2_d = nc.dram_tensor("wout2t", [CL_D[2], CL_SHARD[2]], BF16_DT,
                             kind="ExternalInput")
    wout3_d = nc.dram_tensor("wout3t", [CL_D[3], CL_SHARD[3]], BF16_DT,
                             kind="ExternalInput")
    tix_d = nc.dram_tensor("tgtidx", [128, RT * SW], U16, kind="ExternalInput")
    tgid_d = nc.dram_tensor("tgid", [128, RT, S], BF16_DT, kind="ExternalInput")
    wm_d = nc.dram_tensor("wm", [128, RT, S], BF16_DT, kind="ExternalInput")
    out_d = nc.dram_tensor("out", [1, 1], F32, kind="ExternalOutput")

    EXP = mybir.ActivationFunctionType.Exp
    LN = mybir.ActivationFunctionType.Ln
    ADD = mybir.AluOpType.add
    SUB = mybir.AluOpType.subtract
    MULT = mybir.AluOpType.mult
    ISEQ = mybir.AluOpType.is_equal
    AXX = mybir.AxisListType.X
    DR = mybir.MatmulPerfMode.DoubleRow

    with tile.TileContext(nc) as tc, ExitStack() as ctx:
        sb = ctx.enter_context(tc.tile_pool(name="sb", bufs=1))
        big = ctx.enter_context(tc.tile_pool(name="big", bufs=3))
        ps = ctx.enter_context(tc.tile_pool(name="ps", bufs=2, space="PSUM"))
        dr = ctx.enter_context(tc.tile_pool(name="dr", bufs=1, space="DRAM"))

        # combined exp+ln activation table so EXP and LN interleave with a
        # single table load for the whole kernel
        import os
        if not os.environ.get("ADAK_NO_TABLE_PRELOAD"):
            nc.scalar.add_instruction(mybir.InstLoadActFuncSet(
                name=nc.get_next_instruction_name(),
                act_func_set_id=EXP_TABLE_ID, ins=[], outs=[]))

        # ---- persistent SBUF tensors ----
        xt_sb = sb.tile([128, 8, B], MMDT)             # x.T  [d, b] k-tiled
        whead_sb = sb.tile([128, 8, 1254], MMDT)
        wout0_sb = sb.tile([128, 4, CL_SHARD[0]], MMDT)
        wout1_sb = sb.tile([128, 2, CL_SHARD[1]], MMDT)
        wout2_sb = sb.tile([128, CL_SHARD[2]], BF16_DT)
        wout3_sb = sb.tile([64, CL_SHARD[3]], BF16_DT)
        h0_sb = sb.tile([128, 4, B], MMDT)             # h.T (x S_PROJ)
        h1_sb = sb.tile([128, 2, B], MMDT)
        h2_sb = sb.tile([128, B], BF16_DT)
        h3_sb = sb.tile([64, B], BF16_DT)
        tmpS = sb.tile([128, S], BF16_DT)
        tix_sb = sb.tile([128, RT * SW], U16)
        tgid_sb = sb.tile([128, RT, S], BF16_DT)
        vg3 = sb.tile([128, RT, S], BF16_DT)           # gathered exp(logit)
        wm_sb = sb.tile([128, RT, S], BF16_DT)         # (1-dp)*ownership
        logv3 = sb.tile([128, RT, S], BF16_DT)
        llinkraw = sb.tile([128, RT, 4], F32)          # raw link logits
        zscr = sb.tile([128, 1280], BF16_DT)
        zs = sb.tile([128, RT, 8], F32)   # Z partials per piece, c1/c2/c3
        # AR payload, 11 stats x 8 row-tiles:
        # q: 0 Zh, 1..4 Zc_g, 5 den, 6..9 Wg, 10 numraw
        pay = sb.tile([128, 88], F32)
        rsb = sb.tile([128, 88], F32)
        ones_sb = sb.tile([128, 1], F32)
        out_sb = sb.tile([1, 1], F32)

        pview = pay[:, :].rearrange("p (q r) -> p q r", q=11)
        rview = rsb[:, :].rearrange("p (q r) -> p q r", q=11)

        # ---- input DMAs (order matters: compute-critical tensors first;
        # xt/projt interleaved per k-tile so the h matmuls start early) ----
        pj = sb.tile([128, 8, sum(CL_D)], MMDT)
        xt_r = xt_d.ap().rearrange("(k p) b -> p k b", p=128)
        pj_r = projt_d.ap().rearrange("(k p) c -> p k c", p=128)
        for k in range(8):
            nc.sync.dma_start(out=xt_sb[:, k, :], in_=xt_r[:, k, :])
            nc.sync.dma_start(out=pj[:, k, :], in_=pj_r[:, k, :])
        nc.sync.dma_start(out=whead_sb,
                          in_=whead_d.ap().rearrange("(k p) c -> p k c", p=128))
        nc.sync.dma_start(out=wout0_sb,
                          in_=wout0_d.ap().rearrange("(k p) c -> p k c", p=128))
        nc.sync.dma_start(out=wout1_sb,
                          in_=wout1_d.ap().rearrange("(k p) c -> p k c", p=128))
        nc.sync.dma_start(out=wout2_sb, in_=wout2_d[:])
        nc.sync.dma_start(out=wout3_sb, in_=wout3_d[:])
        nc.sync.dma_start(out=tix_sb, in_=tix_d[:])
        nc.sync.dma_start(out=tgid_sb, in_=tgid_d[:])
        nc.sync.dma_start(out=wm_sb, in_=wm_d[:])

        nc.vector.memset(ones_sb[:, :], 1.0)

        # Prewarm the collectives path: a tiny dummy AllReduce early in the
        # run (nothing in the loop depends on it) absorbs the ~11.5us
        # first-collective ncfw setup and part of the cross-core launch skew,
        # so the real AllReduce at the tail starts almost immediately.
        warm_src = sb.tile([1, 16], F32)
        warm_sb = sb.tile([1, 16], F32)
        warm_in = dr.tile([1, 16], F32)
        warm_out = dr.tile([1, 16], F32, addr_space="Shared")
        nc.vector.memset(warm_src[:, :], 1.0)
        nc.sync.dma_start(out=warm_in, in_=warm_src[:, :])
        nc.gpsimd.collective_compute(
            "AllReduce", ADD, replica_groups=[list(range(NCORES))],
            ins=[warm_in.opt()], outs=[warm_out.opt()])
        warm_anchor = sb.tile([1, 16], F32)
        nc.sync.dma_start(out=warm_sb, in_=warm_out)
        nc.vector.tensor_copy(warm_anchor[:, :], warm_sb[:, :])

        # ---- cluster hidden states h.T (all batch rows, computed locally —
        # a sharded h + AllGather is infeasible: the collectives entry
        # barrier isn't done until ~60-150us into the NEFF) ----
        HT_OFF = [0, 128, 256, 384, 512, 640, 768, 896]
        HT_M = [128, 128, 128, 128, 128, 128, 128, 64]
        for bc in range(2):
            for htile in range(2):
                pst = ps.tile([128, 2048], F32, tag="ps", name=f"hps_{bc}_{htile}")
                for hl in range(4):
                    ht = htile * 4 + hl
                    M = HT_M[ht]
                    if FP8_ON:
                        for kp in range(4):
                            nc.tensor.matmul(
                                pst[0:M, hl * 512:(hl + 1) * 512],
                                pj[:, 2 * kp:2 * kp + 2, HT_OFF[ht]:HT_OFF[ht] + M],
                                xt_sb[:, 2 * kp:2 * kp + 2, bc * 512:(bc + 1) * 512],
                                start=(kp == 0), stop=(kp == 3), perf_mode=DR)
                    else:
                        for k in range(8):
                            nc.tensor.matmul(
                                pst[0:M, hl * 512:(hl + 1) * 512],
                                pj[:, k, HT_OFF[ht]:HT_OFF[ht] + M],
                                xt_sb[:, k, bc * 512:(bc + 1) * 512],
                                start=(k == 0), stop=(k == 7))
                for hl in range(4):
                    ht = htile * 4 + hl
                    src = pst[0:HT_M[ht], hl * 512:(hl + 1) * 512]
                    bsl = slice(bc * 512, (bc + 1) * 512)
                    if ht < 4:
                        nc.scalar.copy(h0_sb[:, ht, bsl], src)
                    elif ht < 6:
                        nc.vector.tensor_copy(h1_sb[:, ht - 4, bsl], src)
                    elif ht == 6:
                        nc.vector.tensor_copy(h2_sb[:, bsl], src)
                    else:
                        nc.vector.tensor_copy(h3_sb[0:64, bsl], src)

        # ---- main loop: logits -> exp (+Z accumulate) -> gather/ln --------
        # groups: (concat_off, width, n_ktiles, fp8, lhsT source, rhs source)
        KW = 2 if FP8_ON else 1

        def lh_head(kp, rt):
            return xt_sb[:, KW * kp:KW * kp + KW, rt * 128:(rt + 1) * 128]

        def lh_c0(kp, rt):
            return h0_sb[:, KW * kp:KW * kp + KW, rt * 128:(rt + 1) * 128]

        def lh_c1(kp, rt):
            return h1_sb[:, KW * kp:KW * kp + KW, rt * 128:(rt + 1) * 128]

        def lh_c2(kp, rt):
            return h2_sb[:, rt * 128:(rt + 1) * 128]

        def lh_c3(kp, rt):
            return h3_sb[0:64, rt * 128:(rt + 1) * 128]

        def rh_head(kp, a, w):
            return whead_sb[:, KW * kp:KW * kp + KW, a:a + w]

        def rh_w0(kp, a, w):
            return wout0_sb[:, KW * kp:KW * kp + KW, a:a + w]

        def rh_w1(kp, a, w):
            return wout1_sb[:, KW * kp:KW * kp + KW, a:a + w]

        def rh_w2(kp, a, w):
            return wout2_sb[:, a:a + w]

        def rh_w3(kp, a, w):
            return wout3_sb[0:64, a:a + w]

        E_HEAD = 1.0 / S_WHEAD
        E_C01 = 1.0 / (S_PROJ * S_WOUT)
        E_C23 = 1.0 / S_PROJ
        if FP8_ON:
            GROUPS = [
                (0, 1254, 4, True, lh_head, rh_head),
                (OFF_CL[0], 1250, 2, True, lh_c0, rh_w0),
                (OFF_CL[1], 2500, 1, True, lh_c1, rh_w1),
                (OFF_CL[2], 5000, 1, False, lh_c2, rh_w2),
                (OFF_CL[3], 2500, 1, False, lh_c3, rh_w3),
            ]
        else:
            GROUPS = [
                (0, 1254, 8, False, lh_head, rh_head),
                (OFF_CL[0], 1250, 4, False, lh_c0, rh_w0),
                (OFF_CL[1], 2500, 2, False, lh_c1, rh_w1),
                (OFF_CL[2], 5000, 1, False, lh_c2, rh_w2),
                (OFF_CL[3], 2500, 1, False, lh_c3, rh_w3),
            ]

        # PSUM piece bounds and the exp segments inside each piece.  The
        # head piece is split in two and pieces are emitted in an order
        # chosen (2-slot pipeline model) so ACT never starves: the PE-heavy
        # head halves hide behind ACT-heavy cluster pieces.
        PB = [0, 1024, 2048, 4096, 6144, 8192, 10240, 12288, CONCAT]
        # (lo, hi, scale, zslot or None)
        SEGS = [
            [(0, 1024, E_HEAD, None)],
            [(1024, 1254, E_HEAD, None), (1254, 2048, E_C01, None)],
            [(2048, 2504, E_C01, None), (2504, 4096, E_C01, 0)],
            [(4096, 5004, E_C01, 1), (5004, 6144, E_C23, 2)],
            [(6144, 8192, E_C23, 3)],
            [(8192, 10004, E_C23, 4), (10004, 10240, E_C23, 5)],
            [(10240, 12288, E_C23, 6)],
            [(12288, CONCAT, E_C23, 7)],
        ]
        ORDER = [7, 4, 1, 2, 0, 6, 3, 5]

        ENDLN = bool(os.environ.get("ADAK_ENDLN"))

        def emit_numer(rt):
            # ln of gathered exp values + weighted-sum numerator
            # (tensor_tensor_reduce faults at runtime on hw — avoid)
            nc.scalar.activation(logv3[:, rt, :], vg3[:, rt, :], LN)
            nc.vector.tensor_tensor(
                tmpS[:, :], logv3[:, rt, :], wm_sb[:, rt, :], MULT)
            nc.vector.tensor_reduce(
                pview[:, 10, rt:rt + 1], tmpS[:, :], AXX, ADD)

        for rt in range(RT):
            expb = big.tile([128, CONCAT_PAD], BF16_DT, tag="big",
                            name=f"expb_{rt}")
            for oi, pi in enumerate(ORDER):
                lo, hi = PB[pi], PB[pi + 1]
                pst = ps.tile([128, hi - lo], F32, tag="ps",
                              name=f"ps_{rt}_{pi}")
                for goff, width, kt, fp8, lh, rh in GROUPS:
                    slo, shi = max(goff, lo), min(goff + width, hi)
                    if slo >= shi:
                        continue
                    # chunks must not cross PSUM bank boundaries (512 f32,
                    # bank grid is piece-relative and pieces are 2048-aligned)
                    subs = []
                    a = slo
                    while a < shi:
                        w = min(shi - a, 512 - ((a - lo) % 512))
                        subs.append((a, w))
                        a += w
                    for kp in range(kt):
                        for a, w in subs:
                            nc.tensor.matmul(
                                pst[:, a - lo:a - lo + w],
                                lh(kp, rt), rh(kp, a - goff, w),
                                start=(kp == 0), stop=(kp == kt - 1),
                                perf_mode=DR if fp8 else None)
                for slo, shi, sc, zslot in SEGS[pi]:
                    acc = (zs[:, rt, zslot:zslot + 1]
                           if zslot is not None else None)
                    nc.scalar.activation(
                        expb[:, slo:shi], pst[:, slo - lo:shi - lo], EXP,
                        scale=sc, accum_out=acc)
                if pi == 1:
                    # raw link logits out of PSUM (their ln IS the logit)
                    nc.vector.tensor_scalar(
                        llinkraw[:, rt, :], pst[:, 1250 - lo:1254 - lo],
                        E_HEAD, None, op0=MULT)
                if oi == 3 and rt > 0 and not ENDLN:
                    emit_numer(rt - 1)
            # head/c0 Z partial sums on DVE (head must exclude link columns)
            for q, (za, zw) in enumerate(
                    [(0, SH_SHARD), (OFF_CL[0], CL_SHARD[0])]):
                nc.vector.tensor_scalar(
                    zscr[:, 0:zw],
                    expb[:, za:za + zw], 1.0, 0.0, op0=MULT, op1=ADD,
                    accum_out=pview[:, q, rt:rt + 1])
            # gather exp(logit) at this core's targets
            nc.gpsimd.indirect_copy(
                vg3[:, rt, :], expb[:, 0:CONCAT],
                tix_sb[:, rt * SW:(rt + 1) * SW], True)
            # per-row-tile statistics that don't need log(v): den, W_g
            # (Wsl is never needed: den already equals Wsl + sum_g Wg)
            nc.vector.tensor_reduce(
                pview[:, 5, rt:rt + 1], wm_sb[:, rt, :], AXX, ADD)
            for gi in range(1, 5):
                nc.vector.scalar_tensor_tensor(
                    tmpS[:, :], tgid_sb[:, rt, :], float(gi + 1), wm_sb[:, rt, :],
                    op0=ISEQ, op1=MULT)
                nc.vector.tensor_reduce(
                    pview[:, 5 + gi, rt:rt + 1], tmpS[:, :], AXX, ADD)
        if ENDLN:
            nc.scalar.activation(
                logv3[:, :, :].rearrange("p a b -> p (a b)"),
                vg3[:, :, :].rearrange("p a b -> p (a b)"), LN)
            tmp3 = vg3
            nc.vector.tensor_tensor(
                tmp3[:, :, :], logv3[:, :, :], wm_sb[:, :, :], MULT)
            nc.vector.tensor_reduce(pview[:, 10, :], tmp3[:, :, :], AXX, ADD)
        else:
            emit_numer(RT - 1)

        # combine the c1/c2/c3 Z piece-partials into the payload
        t8z = sb.tile([128, 8], F32)
        nc.vector.tensor_tensor(pview[:, 2, :], zs[:, :, 0], zs[:, :, 1], ADD)
        nc.vector.tensor_tensor(t8z[:, :], zs[:, :, 2], zs[:, :, 3], ADD)
        nc.vector.tensor_tensor(pview[:, 3, :], t8z[:, :], zs[:, :, 4], ADD)
        nc.vector.tensor_tensor(t8z[:, :], zs[:, :, 5], zs[:, :, 6], ADD)
        nc.vector.tensor_tensor(pview[:, 4, :], t8z[:, :], zs[:, :, 7], ADD)

        # ---- AllReduce the statistics ----
        cc_in = dr.tile([128, 88], F32)
        cc_out = dr.tile([128, 88], F32, addr_space="Shared")
        nc.sync.dma_start(out=cc_in, in_=pay[:, :])
        nc.gpsimd.collective_compute(
            "AllReduce", ADD,
            replica_groups=[list(range(NCORES))],
            ins=[cc_in.opt()], outs=[cc_out.opt()])
        nc.sync.dma_start(out=rsb, in_=cc_out)

        # ---- final combine (identical on every core) ----
        linkexp = sb.tile([128, 32], F32)
        lsum = sb.tile([128, 8], F32)
        zcomb = sb.tile([128, 40], F32)
        lnz = sb.tile([128, 40], F32)
        s8 = sb.tile([128, 8], F32)
        tA = sb.tile([128, 8], F32)
        num8 = sb.tile([128, 8], F32)
        rden = sb.tile([128, 8], F32)
        pcol = sb.tile([128, 1], F32)
        llview = llinkraw[:, :, :]

        nc.scalar.activation(
            linkexp[:, :], llview.rearrange("p a b -> p (a b)"), EXP)
        lexp3 = linkexp[:, :].rearrange("p (r g) -> p r g", g=4)
        nc.vector.tensor_reduce(lsum[:, :], lexp3, AXX, ADD)
        nc.vector.tensor_tensor(zcomb[:, 0:8], rview[:, 0, :], lsum[:, :], ADD)
        nc.vector.tensor_copy(zcomb[:, 8:40], rsb[:, 8:40])
        nc.scalar.activation(lnz[:, :], zcomb[:, :], LN)
        llink3 = llview.rearrange("p r g -> p g r")
        for g in range(4):
            nc.vector.tensor_tensor(
                tA[:, :], llink3[:, g, :], lnz[:, 8 + 8 * g:16 + 8 * g], SUB)
            if g == 0:
                nc.vector.tensor_tensor(s8[:, :], tA[:, :], rview[:, 6 + g, :], MULT)
            else:
                nc.vector.tensor_tensor(tA[:, :], tA[:, :], rview[:, 6 + g, :], MULT)
                nc.vector.tensor_tensor(s8[:, :], s8[:, :], tA[:, :], ADD)
        # num = numraw + s8 - den * logZh
        nc.vector.tensor_tensor(tA[:, :], rview[:, 5, :], lnz[:, 0:8], MULT)
        nc.vector.tensor_tensor(num8[:, :], rview[:, 10, :], tA[:, :], SUB)
        nc.vector.tensor_tensor(num8[:, :], num8[:, :], s8[:, :], ADD)
        nc.vector.reciprocal(rden[:, :], rview[:, 5, :])
        nc.vector.tensor_tensor(num8[:, :], num8[:, :], rden[:, :], MULT)
        nc.vector.tensor_reduce(pcol[:, :], num8[:, :], AXX, ADD)
        psq = ps.tile([1, 1], F32, tag="ps")
        nc.tensor.matmul(psq[0:1, 0:1], pcol[:, 0:1], ones_sb[:, 0:1],
                         start=True, stop=True)
        nc.scalar.mul(out_sb[:, :], psq[0:1, 0:1], -1.0 / (B + 1e-5))
        nc.sync.dma_start(out=out_d[:], in_=out_sb)

    nc.compile()
    _CACHE[S] = nc
    return nc


# ----------------------------------------------------------------------------
# host-side sharding / index routing
# ----------------------------------------------------------------------------


def _f8(a, scale):
    return np.clip(np.asarray(a, np.float32) * scale, -239.0, 239.0).astype(F8)


def _shard_inputs(features, head_weight, projs, outs, discard_probs,
                  targets, target_mask):
    """Build the 8 per-core input maps. Returns (in_maps, S)."""
    if FP8_ON:
        xt = _f8(np.ascontiguousarray(features.T), 1.0)
        projt = _f8(np.concatenate([p.T for p in projs], axis=1), S_PROJ)
    else:
        xt = np.ascontiguousarray(features.T).astype(BF16)
        projt = (np.concatenate([p.T for p in projs], axis=1)
                 * S_PROJ).astype(BF16)

    tgt = np.asarray(targets).astype(np.int64).reshape(-1)
    msk = np.asarray(target_mask).astype(bool).reshape(-1)
    bb = np.repeat(np.arange(B, dtype=np.int64), T)

    grp = np.digitize(tgt, GRP_BOUNDS[1:-1])          # 0..4 (0 = shortlist)
    u = tgt - np.asarray(GRP_BOUNDS)[grp]
    shard = np.asarray(GRP_SHARD)[grp]
    core = u // shard
    jcat = u % shard + np.asarray(GRP_OFF)[grp]
    wval = (1.0 - discard_probs[tgt]).astype(np.float32)

    rt = bb >> 7
    gc = (bb >> 4) & 7

    # padded slots per (core, rt, gc)
    key_all = ((core * RT + rt) * 8 + gc).astype(np.int64)
    valid = msk
    counts = np.bincount(key_all[valid], minlength=NCORES * RT * 8)
    # multiple of 32 so each row-tile's wrapped idx slice stays 4B-aligned
    S = int(counts.max())
    S = ((S + 31) // 32) * 32

    in_maps = []
    for c in range(NCORES):
        sel = valid & (core == c)
        jj = jcat[sel]
        bsel = bb[sel]
        gsel = grp[sel]
        rts = rt[sel]
        gcs = gc[sel]
        ww = wval[sel]
        po = bsel & 15
        key = rts * 8 + gcs
        order = np.argsort(key, kind="stable")
        jj, bsel, gsel, rts, gcs, po, ww = (a[order] for a in
                                            (jj, bsel, gsel, rts, gcs, po, ww))
        key = key[order]
        # slot within each (rt, gc) bucket
        start_of = np.r_[0, np.flatnonzero(np.diff(key)) + 1]
        bucket_len = np.diff(np.r_[start_of, len(key)])
        slot = np.arange(len(key)) - np.repeat(start_of, bucket_len)

        tix = np.full((128, RT * (S // 16)), PADIDX, np.uint16)
        tix[16 * gcs + slot % 16, rts * (S // 16) + slot // 16] = jj.astype(np.uint16)
        tgid = np.zeros((128, RT, S), np.float32)
        tgid[16 * gcs + po, rts, slot] = gsel + 1.0
        tgid = tgid.astype(BF16)
        wm = np.zeros((128, RT, S), np.float32)
        wm[16 * gcs + po, rts, slot] = ww
        wm = wm.astype(BF16)

        # head shard + link columns, transposed
        hslice = head_weight[c * SH_SHARD:(c + 1) * SH_SHARD]
        wh_cat = np.concatenate(
            [hslice.T, head_weight[SHORT:SHORT + 4].T], axis=1)
        wheadt = (_f8(wh_cat, S_WHEAD) if FP8_ON
                  else (wh_cat * S_WHEAD).astype(BF16))
        in_maps.append({
            "xt": xt,
            "projt": projt,
            "wheadt": wheadt,
            "wout0t": (_f8(outs[0][c * CL_SHARD[0]:(c + 1) * CL_SHARD[0]].T,
                           S_WOUT) if FP8_ON else
                       (outs[0][c * CL_SHARD[0]:(c + 1) * CL_SHARD[0]].T
                        * S_WOUT).astype(BF16)),
            "wout1t": (_f8(outs[1][c * CL_SHARD[1]:(c + 1) * CL_SHARD[1]].T,
                           S_WOUT) if FP8_ON else
                       (outs[1][c * CL_SHARD[1]:(c + 1) * CL_SHARD[1]].T
                        * S_WOUT).astype(BF16)),
            "wout2t": np.ascontiguousarray(
                outs[2][c * CL_SHARD[2]:(c + 1) * CL_SHARD[2]].T).astype(BF16),
            "wout3t": np.ascontiguousarray(
                outs[3][c * CL_SHARD[3]:(c + 1) * CL_SHARD[3]].T).astype(BF16),
            "tgtidx": tix,
            "tgid": tgid,
            "wm": wm,
        })
    return in_maps, S


def _run(features, head_weight, proj0, out0, proj1, out1, proj2, out2,
         proj3, out3, discard_probs, targets, target_mask,
         trace=False, tmpdir=None):
    features = np.asarray(features, np.float32)
    head_weight = np.asarray(head_weight, np.float32)
    projs = [np.asarray(p, np.float32) for p in (proj0, proj1, proj2, proj3)]
    outs = [np.asarray(o, np.float32) for o in (out0, out1, out2, out3)]
    discard_probs = np.asarray(discard_probs, np.float32)

    in_maps, S = _shard_inputs(features, head_weight, projs, outs,
                               discard_probs, targets, target_mask)
    nc = _build(S)
    res = run_bass_kernel_spmd(nc, in_maps, list(range(NCORES)),
                               trace=trace, tmpdir=tmpdir)
    val = np.asarray(res.results[0]["out"], np.float32).reshape(())
    return val, res


def kernel(**inputs) -> np.ndarray:
    val, _ = _run(**inputs)
    return val
